# revision 29
# baseline (speedup 1.0000x reference)
"""Trainium2 Bass kernel for nn_CapsuleLayer (capsule dynamic routing).

Math (reference):
    u_hat[b,c,u,s] = sum_i W[c,u,s,i] * x[b,i,c]          (never materialized here)
    3 routing iterations:
        c_ij = softmax_u(b_ij)                            [C, U]
        s_j[b,u,s]  = sum_c c_ij[c,u] * u_hat[b,c,u,s]
        v_j = squash(s_j)   (norm over the U axis!)
        u_vj1[c,u] = sum_{b,s} u_hat[b,c,u,s] v_j[b,u,s] / B
        b_ij += u_vj1
    output = v_j  (B, U, S, 1)

Sharding: channels C=1152 split 8 ways (CL=144 per core).  Per core both
x-slice and W-slice live in SBUF, so u_hat is recomputed on the fly as
matrix products (contraction over (c,i)=2304 or over b=256), all shapes
128-partition friendly.  The only cross-core data dependency is the
s_j partial sum: one 320 KB collective per routing iteration.

Inputs x/W are staged in f16 (matmul + elementwise operands); all
accumulations (PSUM), squash and softmax stay fp32.

Per-core layouts (host-prepared):
    XT [128, T*B ] f16 : XT[p, t*256+b]      = x[b, i, c],  ci = 128t+p
    XF [128, 2*KCI] f16: XF[p, bc*2304+ci]   = x[b, i, c],  b  = 128bc+p
    WM [128, T*US] f16 : WM[p, t*320+u*32+s] = W[c, u, s, i], ci = 128t+p
    EM [128, 128] f32   : block-diag selector, EM[p,m] = (p//16==m//16)/256
Column convention for the (u,s) axis everywhere: col = u*32 + s
(u-major, so the routing-update reduce over s is contiguous).
"""

import numpy as np

B, IN_U, C, NUM_U, S = 256, 16, 1152, 10, 32
NCORES = 8
CL = C // NCORES          # 144 channels per core
KCI = CL * IN_U           # 2304 contraction size
T = KCI // 128            # 18 partition chunks
US = NUM_U * S            # 320
NITER = 3
G = 3                     # chunk groups (pipelining granularity)
CPG = T // G              # 6 chunks per group

_CACHE = {}


def _build_program(bypass_cc=False, n_reps=1, host_final=True):
    import concourse.bacc as bacc
    import concourse.tile as tile
    from concourse import mybir
    from contextlib import ExitStack

    f32 = mybir.dt.float32
    f16 = mybir.dt.float16
    AX = mybir.AxisListType
    ALU = mybir.AluOpType
    AF = mybir.ActivationFunctionType

    # Bacc (not raw Bass): its compile() pipeline legalizes multi-wait
    # instructions (move_matmul_waits_to_ldweights + generate_event_semaphores)
    # which walrus codegen otherwise rejects ("Too many sync wait commands").
    nc = bacc.Bacc(None, num_devices=NCORES)
    xt_d = nc.declare_dram_parameter("xt", [128, T * B], f16, isOutput=False)
    xf_d = nc.declare_dram_parameter("xf", [128, 2 * KCI], f16, isOutput=False)
    wm_d = nc.declare_dram_parameter("wm", [128, T * US], f16, isOutput=False)
    em_d = nc.declare_dram_parameter("em", [128, 128], f32, isOutput=False)
    out_d = nc.declare_dram_parameter("out", [128, 2 * US], f32, isOutput=True)

    with tile.TileContext(nc) as tc, ExitStack() as ctx:
        singles = ctx.enter_context(tc.tile_pool(name="singles", bufs=1))
        big = ctx.enter_context(tc.tile_pool(name="big", bufs=1))
        work = ctx.enter_context(tc.tile_pool(name="work", bufs=2))
        qwork = ctx.enter_context(tc.tile_pool(name="qwork", bufs=3))
        psum_s = ctx.enter_context(tc.tile_pool(name="psum_s", bufs=1, space="PSUM"))
        psum_m = ctx.enter_context(tc.tile_pool(name="psum_m", bufs=2, space="PSUM"))
        psum_u = ctx.enter_context(tc.tile_pool(name="psum_u", bufs=1, space="PSUM"))
        dram = ctx.enter_context(tc.tile_pool(name="dram", bufs=2, space="DRAM"))

        xt_sb = singles.tile([128, T * B], f16, name="xt_sb")
        wm_sb = singles.tile([128, T * US], f16, name="wm_sb")
        xf_sb = singles.tile([128, 2 * KCI], f16, name="xf_sb")
        em_sb = singles.tile([128, 128], f32, name="em_sb")
        bij_sb = singles.tile([128, T * NUM_U], f32, name="bij_sb")

        def load_inputs():
            # Spread across engine queues so the transfers overlap: xt on SP,
            # wm split Pool/Act, xf trails on Act/SP, em on Pool (all idle at
            # rep start; mm1 iter0 needs xt+wm pairs first).
            wm_eng = [nc.gpsimd, nc.scalar, nc.gpsimd]
            for g in range(G):
                nc.sync.dma_start(
                    out=xt_sb[:, g * CPG * B : (g + 1) * CPG * B],
                    in_=xt_d[:, g * CPG * B : (g + 1) * CPG * B],
                )
                wm_eng[g].dma_start(
                    out=wm_sb[:, g * CPG * US : (g + 1) * CPG * US],
                    in_=wm_d[:, g * CPG * US : (g + 1) * CPG * US],
                )
            nc.scalar.dma_start(out=xf_sb[:, 0:KCI], in_=xf_d[:, 0:KCI])
            nc.sync.dma_start(out=xf_sb[:, KCI : 2 * KCI], in_=xf_d[:, KCI : 2 * KCI])
            nc.gpsimd.dma_start(out=em_sb, in_=em_d[:])

        def mm1(rhs_groups, scale, out_sb=None, gsize=CPG):
            """s_partial[b,(u,s)] = XT.T @ rhs, scaled; -> [128, 2*US].

            bc-outer so half 0's PSUM->SBUF copy (and its collective-input
            DMA) overlaps half 1's matmuls.
            """
            cc_sb = out_sb
            if cc_sb is None:
                cc_sb = work.tile([128, 2 * US], f16, name="cc_sb")
            for bc in range(2):
                ps = psum_s.tile([128, US], f32, name=f"s_ps{bc}")
                for t in range(T):
                    rhs = rhs_groups[t // gsize]
                    tl = t % gsize
                    nc.tensor.matmul(
                        ps,
                        lhsT=xt_sb[:, t * B + bc * 128 : t * B + bc * 128 + 128],
                        rhs=rhs[:, tl * US : (tl + 1) * US],
                        start=(t == 0),
                        stop=(t == T - 1),
                    )
                nc.scalar.activation(
                    out=cc_sb[:, bc * US : (bc + 1) * US],
                    in_=ps,
                    func=AF.Copy,
                    scale=float(scale),
                )
            return cc_sb

        def allreduce(cc_sb):
            cc_in = dram.tile([128, 2 * US], f16, name="cc_in")
            cc_out = dram.tile([128, 2 * US], f16, name="cc_out")
            for bc in range(2):
                nc.gpsimd.dma_start(
                    out=cc_in[:, bc * US : (bc + 1) * US],
                    in_=cc_sb[:, bc * US : (bc + 1) * US],
                )
            if bypass_cc:
                nc.gpsimd.dma_start(out=cc_out, in_=cc_in)
            else:
                nc.gpsimd.collective_compute(
                    "AllReduce",
                    ALU.add,
                    replica_groups=[list(range(NCORES))],
                    ins=[cc_in.opt()],
                    outs=[cc_out.opt()],
                )
            s_sb = work.tile([128, 2 * US], f16, name="s_sb")
            for bc in range(2):
                nc.gpsimd.dma_start(
                    out=s_sb[:, bc * US : (bc + 1) * US],
                    in_=cc_out[:, bc * US : (bc + 1) * US],
                )
            return s_sb

        PR = 128 // NCORES  # 16 partition rows per core after ReduceScatter

        def reduce_scatter(cc_sb):
            """Final iteration: each core only needs its 1/8 of s_j."""
            cc_in = dram.tile([128, 2 * US], f16, name="rs_in")
            cc_out = dram.tile([PR, 2 * US], f16, name="rs_out")
            nc.gpsimd.dma_start(out=cc_in, in_=cc_sb)
            if bypass_cc:
                nc.gpsimd.dma_start(out=cc_out, in_=cc_in[0:PR, :])
            else:
                nc.gpsimd.collective_compute(
                    "ReduceScatter",
                    ALU.add,
                    replica_groups=[list(range(NCORES))],
                    ins=[cc_in.opt()],
                    outs=[cc_out.opt()],
                )
            s16 = work.tile([PR, 2 * US], f16, name="s16")
            nc.gpsimd.dma_start(out=s16, in_=cc_out)
            return s16

        def squash(s_sb, rows=128, out_f32=False):
            """v = s * mag/(1+mag^2), mag^2 summed over u per (b, s').

            All on DVE except the tiny Sqrt (Act).  Column layout (bc,u,s):
            the u-sum is strided (stride S), 640 elems.
            """
            sq = work.tile([rows, 2 * US], f32, name="sq")
            magsq = work.tile([rows, 2 * S], f32, name="magsq")
            for bc in range(2):
                nc.vector.tensor_mul(
                    out=sq[:, bc * US : (bc + 1) * US],
                    in0=s_sb[:, bc * US : (bc + 1) * US],
                    in1=s_sb[:, bc * US : (bc + 1) * US],
                )
                nc.vector.reduce_sum(
                    out=magsq[:, bc * S : (bc + 1) * S],
                    in_=sq[:, bc * US : (bc + 1) * US]
                    .rearrange("p (u s) -> p u s", u=NUM_U)
                    .transpose([0, 2, 1]),
                    axis=AX.X,
                )
            mag = work.tile([rows, 2 * S], f32, name="mag")
            nc.scalar.sqrt(out=mag, in_=magsq)
            den = work.tile([rows, 2 * S], f32, name="den")
            nc.vector.tensor_scalar_add(out=den, in0=magsq, scalar1=1.0)
            rden = work.tile([rows, 2 * S], f32, name="rden")
            nc.vector.reciprocal(out=rden, in_=den)
            fct = work.tile([rows, 2 * S], f32, name="fct")
            nc.vector.tensor_mul(out=fct, in0=mag, in1=rden)
            v_sb = work.tile([rows, 2 * US], f32 if out_f32 else f16, name="v_sb")
            for bc in range(2):
                nc.vector.tensor_mul(
                    out=v_sb[:, bc * US : (bc + 1) * US].rearrange(
                        "p (u s) -> p u s", u=NUM_U
                    ),
                    in0=s_sb[:, bc * US : (bc + 1) * US].rearrange(
                        "p (u s) -> p u s", u=NUM_U
                    ),
                    in1=fct[:, bc * S : (bc + 1) * S]
                    .unsqueeze(1)
                    .broadcast_to([rows, NUM_U, S]),
                )
            return v_sb

        def routing_update(v_bf, first):
            """u_vj1 -> b_ij update -> softmax; returns c_ij (f16).

            Per-t pipeline: PE (2 matmuls, contract over b) -> Pool multiplies
            wm against the PSUM result directly -> DVE reduces over s
            (contiguous).  No scalar-engine copies.
            """
            r_sb = work.tile([128, T * NUM_U], f32, name="r_sb")
            # Pairs of t-chunks: each matmul lands in its own PSUM bank of a
            # 2-bank tile (offsets 0 and 512 f32), so one Act copy moves both
            # to SBUF (GPSIMD cannot read PSUM; DVE doing it would bottleneck).
            for tp in range(T // 2):
                ps = psum_m.tile([128, 1024], f32, name="m_ps")
                for half in range(2):
                    t = 2 * tp + half
                    dst = ps[:, half * 512 : half * 512 + US]
                    for bc in range(2):
                        nc.tensor.matmul(
                            dst,
                            lhsT=xf_sb[
                                :, bc * KCI + t * 128 : bc * KCI + (t + 1) * 128
                            ],
                            rhs=v_bf[:, bc * US : (bc + 1) * US],
                            start=(bc == 0),
                            stop=(bc == 1),
                        )
                m2 = qwork.tile([128, 2 * US], f16, name="m2")
                nc.scalar.copy(
                    out=m2.rearrange("p (h c) -> p h c", h=2),
                    in_=ps.rearrange("p (h c) -> p h c", h=2)[:, :, 0:US],
                )
                q2 = qwork.tile([128, 2 * US], f16, name="q2")
                nc.gpsimd.tensor_mul(
                    out=q2, in0=wm_sb[:, 2 * tp * US : (2 * tp + 2) * US], in1=m2
                )
                nc.vector.reduce_sum(
                    out=r_sb[:, 2 * tp * NUM_U : (2 * tp + 2) * NUM_U],
                    in_=q2.rearrange("p (t u s) -> p t u s", t=2, u=NUM_U),
                    axis=AX.X,
                )
            ups = psum_u.tile([128, T * NUM_U], f32, name="u_ps")
            nc.tensor.matmul(ups, lhsT=em_sb, rhs=r_sb, start=True, stop=True)
            # softmax over u (inner groups of 10); b_ij stays small (<9), so
            # exp needs no max-subtraction.
            ex = work.tile([128, T * NUM_U], f32, name="ex")
            if first:
                nc.vector.tensor_copy(out=bij_sb, in_=ups)
                nc.scalar.activation(out=ex, in_=ups, func=AF.Exp)
            else:
                nc.vector.tensor_add(out=bij_sb, in0=bij_sb, in1=ups)
                nc.scalar.activation(out=ex, in_=bij_sb, func=AF.Exp)
            sm = work.tile([128, T], f32, name="sm")
            nc.vector.reduce_sum(
                out=sm, in_=ex.rearrange("p (t u) -> p t u", t=T), axis=AX.X
            )
            rsm = work.tile([128, T], f32, name="rsm")
            nc.vector.reciprocal(out=rsm, in_=sm)
            cij_sb = work.tile([128, T * NUM_U], f16, name="cij_sb")
            nc.vector.tensor_mul(
                out=cij_sb.rearrange("p (t u) -> p t u", t=T),
                in0=ex.rearrange("p (t u) -> p t u", t=T),
                in1=rsm[:].unsqueeze(2).broadcast_to([128, T, NUM_U]),
            )
            return cij_sb

        BMG = 6                  # bm chunks (finer than G for mm1 overlap)
        CP2 = T // BMG

        def bm_build(cij_sb):
            groups = []
            for g in range(BMG):
                # Alternate engines so the muls overlap (stride-0 innermost
                # broadcast runs the slow DVE path; Pool is a bit faster and
                # gets the even chunks, including mm1's first operand).
                eng = nc.gpsimd if g % 2 == 0 else nc.vector
                bm_g = big.tile([128, CP2 * US], f16, name=f"bm_g{g}")
                eng.tensor_mul(
                    out=bm_g.rearrange("p (t u s) -> p t u s", t=CP2, u=NUM_U),
                    in0=wm_sb[:, g * CP2 * US : (g + 1) * CP2 * US].rearrange(
                        "p (t u s) -> p t u s", t=CP2, u=NUM_U
                    ),
                    in1=cij_sb[:, g * CP2 * NUM_U : (g + 1) * CP2 * NUM_U]
                    .rearrange("p (t u) -> p t u", t=CP2)
                    .unsqueeze(3)
                    .broadcast_to([128, CP2, NUM_U, S]),
                )
                groups.append(bm_g)
            return groups

        wm_groups = [
            wm_sb[:, g * CPG * US : (g + 1) * CPG * US] for g in range(G)
        ]
        for _rep in range(n_reps):
            load_inputs()
            v_bf = None
            for it in range(NITER):
                if it == 0:
                    cc = mm1(wm_groups, 1.0 / NUM_U)
                    v_bf = squash(allreduce(cc))
                elif it < NITER - 1:
                    cij = routing_update(v_bf, first=(it == 1))
                    cc = mm1(bm_build(cij), 1.0, gsize=CP2)
                    v_bf = squash(allreduce(cc))
                elif host_final:
                    # Final iteration: emit the per-core PARTIAL s_j in f32.
                    # The cross-core sum + squash happen on the host as part
                    # of the gather/unshard step (postprocess).
                    cij = routing_update(v_bf, first=(it == 1))
                    out_sb = work.tile([128, 2 * US], f32, name="out_sb")
                    mm1(bm_build(cij), 1.0, out_sb=out_sb, gsize=CP2)
                    for bc in range(2):
                        nc.sync.dma_start(
                            out=out_d[:, bc * US : (bc + 1) * US],
                            in_=out_sb[:, bc * US : (bc + 1) * US],
                        )
                else:
                    cij = routing_update(v_bf, first=(it == 1))
                    cc = mm1(bm_build(cij), 1.0)
                    s16 = reduce_scatter(cc)
                    v16 = squash(s16, rows=PR, out_f32=True)
                    nc.sync.dma_start(out=out_d[0:PR, :], in_=v16)

    return nc


def _prep_core_inputs(x, W, core, em):
    sl = slice(core * CL, (core + 1) * CL)
    xs = np.ascontiguousarray(x[:, :, sl])  # (B, I, CL)
    ws = np.ascontiguousarray(W[0, sl])     # (CL, U, S, I)
    xt = xs.transpose(2, 1, 0).reshape(T, 128, B)
    xt = np.ascontiguousarray(xt.transpose(1, 0, 2)).reshape(128, T * B)
    xf = xs.transpose(0, 2, 1).reshape(2, 128, KCI)
    xf = np.ascontiguousarray(xf.transpose(1, 0, 2)).reshape(128, 2 * KCI)
    wm = ws.transpose(0, 3, 1, 2).reshape(T, 128, US)  # (c,i),(u,s) u-major
    wm = np.ascontiguousarray(wm.transpose(1, 0, 2)).reshape(128, T * US)
    return {
        "xt": xt.astype(np.float16),
        "xf": xf.astype(np.float16),
        "wm": wm.astype(np.float16),
        "em": em,
    }


def prep_in_maps(x, W):
    x = np.asarray(x, dtype=np.float32)
    W = np.asarray(W, dtype=np.float32)
    em = (np.kron(np.eye(8, dtype=np.float32), np.ones((16, 16), np.float32))
          / float(B))
    return [_prep_core_inputs(x, W, core, em) for core in range(NCORES)]


def postprocess(results):
    """Sum the per-core partial s_j [128, 640] (col = bc*320 + u*32 + s),
    apply the final squash, then -> (B, U, S, 1)."""
    s = np.zeros((128, 2 * US), np.float32)
    for r in range(NCORES):
        s += np.asarray(results[r]["out"], np.float32)
    s = s.reshape(128, 2, NUM_U, S).transpose(1, 0, 2, 3).reshape(B, NUM_U, S)
    mag_sq = np.sum(s * s, axis=1, keepdims=True)
    v = s * (mag_sq / ((1.0 + mag_sq) * np.sqrt(mag_sq)))
    return np.ascontiguousarray(v[..., None])


def get_program(bypass_cc=False, n_reps=1, host_final=True):
    key = ("nc", bypass_cc, n_reps, host_final)
    if key not in _CACHE:
        nc = _build_program(bypass_cc=bypass_cc, n_reps=n_reps,
                            host_final=host_final)
        nc.finalize()  # runs Bacc.compile(): reg alloc + sync-wait legalization
        _CACHE[key] = nc
    return _CACHE[key]


def kernel(x, W):
    from concourse.bass_utils import run_bass_kernel_spmd

    nc = get_program()
    in_maps = prep_in_maps(x, W)
    res = run_bass_kernel_spmd(nc, in_maps, list(range(NCORES)))
    return postprocess(res.results)
